# revision 9
# baseline (speedup 1.0000x reference)
"""DeepseekV3 MoE (calibrate) Trainium2 kernel.

Strategy (8 NeuronCores, SPMD via run_bass_kernel_spmd):
  - Expert-parallel: 4 of 32 experts per core; shared expert tensor-parallel
    (intermediate dim 1536 -> 192 per core). Host sums the 8 partial outputs.
  - Gate computed on every core in full fp32 (selection-exactness); gate_w
    columns are host-permuted per core so the core's own 4 experts are always
    columns 0..3 (keeps the program SPMD-uniform).
  - Top-6 + combine weights on DVE (iterated max over [128, 8, 32] logits).
  - Capacity-based token compaction (C=256 slots/expert): slot index per token
    via triangular-matmul cumsum; one-hot gather/scatter matrices built with
    iota + is_equal; gather/scatter run as float32r matmuls (1 cyc/row,
    ~1e-4 relative rounding).
  - Expert MLPs + shared expert in float32r. PSUM accumulates fp32.
  - Per-expert down-proj outputs (Y) spill to DRAM tiles and stream back in a
    final d-tile loop that accumulates routed+shared into token-major output.
"""

import sys

if "/opt/trn_rl_repo" not in sys.path:
    sys.path.insert(0, "/opt/trn_rl_repo")

from contextlib import ExitStack

import numpy as np

import concourse.bass as bass
import concourse.tile as tile
from concourse import bacc, mybir

dt = mybir.dt
AF = mybir.ActivationFunctionType
ALU = mybir.AluOpType
AX = mybir.AxisListType

T, D, E, F = 1024, 2048, 32, 768
ELOC, C, K = 4, 256, 6
FSH, FSHL = 1536, 192
TCH, DCH, FCH = T // 128, D // 128, F // 128  # 8, 16, 6
NCORES = 8

_COMPILED = None


def _build():
    nc = bacc.Bacc("TRN2", target_bir_lowering=False, debug=False)

    f32, f32r = dt.float32, dt.float32r
    xtok_h = nc.declare_dram_parameter("xtok", [128, TCH, D], f32r, isOutput=False)
    xT_h = nc.declare_dram_parameter("xT", [128, DCH, T], f32, isOutput=False)
    gw_h = nc.declare_dram_parameter("gw", [128, DCH, E], f32, isOutput=False)
    wg_h = nc.declare_dram_parameter("wg", [ELOC, FCH, 128, DCH, 128], f32r, isOutput=False)
    wu_h = nc.declare_dram_parameter("wu", [ELOC, FCH, 128, DCH, 128], f32r, isOutput=False)
    wd_h = nc.declare_dram_parameter("wd", [ELOC, 4, 128, FCH, 512], f32r, isOutput=False)
    sg_h = nc.declare_dram_parameter("sg", [128, DCH, FSHL], f32r, isOutput=False)
    su_h = nc.declare_dram_parameter("su", [128, DCH, FSHL], f32r, isOutput=False)
    sd0_h = nc.declare_dram_parameter("sd0", [128, D], f32r, isOutput=False)
    sd1_h = nc.declare_dram_parameter("sd1", [64, D], f32r, isOutput=False)
    out_h = nc.declare_dram_parameter("out", [T, D], f32, isOutput=True)

    with tile.TileContext(nc) as tc, ExitStack() as ctx:
        pers = ctx.enter_context(tc.tile_pool(name="pers", bufs=1))
        ps_mx = ctx.enter_context(tc.tile_pool(name="ps_mx", bufs=2, space="PSUM"))
        ps_gu = ctx.enter_context(tc.tile_pool(name="ps_gu", bufs=4, space="PSUM"))
        ps_y = ctx.enter_context(tc.tile_pool(name="ps_y", bufs=2, space="PSUM"))
        dram = ctx.enter_context(tc.tile_pool(name="dram", bufs=1, space="DRAM"))

        # ---- constants ----
        iotaC = pers.tile([128, C], f32, tag="iotaC")
        nc.gpsimd.iota(iotaC[:], pattern=[[1, C]], channel_multiplier=0,
                       allow_small_or_imprecise_dtypes=True)
        iotaP = pers.tile([128, 1], f32, tag="iotaP")
        nc.gpsimd.iota(iotaP[:], pattern=[[0, 1]], channel_multiplier=1,
                       allow_small_or_imprecise_dtypes=True)
        iotaR = pers.tile([128, 128], f32, tag="iotaR")
        nc.gpsimd.iota(iotaR[:], pattern=[[1, 128]], channel_multiplier=0,
                       allow_small_or_imprecise_dtypes=True)
        ident = pers.tile([128, 128], f32r, tag="ident")
        nc.vector.tensor_scalar(ident[:], iotaR[:], iotaP[:, 0:1], None, op0=ALU.is_equal)
        ustrict = pers.tile([128, 128], f32r, tag="ustrict")
        nc.vector.tensor_scalar(ustrict[:], iotaR[:], iotaP[:, 0:1], None, op0=ALU.is_gt)
        ones128 = pers.tile([128, 128], f32r, tag="ones128")
        nc.vector.tensor_scalar(ones128[:], iotaR[:], -1.0, None, op0=ALU.is_ge)

        # ---- persistent data ----
        gw = pers.tile([128, DCH, E], f32, tag="gw")
        nc.sync.dma_start(gw[:], gw_h[:])
        xtok = pers.tile([128, TCH, D], f32r, tag="xtok")
        nc.sync.dma_start(xtok[:], xtok_h[:])

        logits = pers.tile([128, TCH, E], f32, tag="logits")
        sel = pers.tile([128, TCH, E], f32, tag="sel")
        wfull = pers.tile([128, TCH, E], f32, tag="wfull")
        wr = pers.tile([128, TCH, E], f32r, tag="wr")
        selr = pers.tile([128, TCH, ELOC], f32r, tag="selr")
        pos_sel = pers.tile([128, TCH, ELOC], f32, tag="pos_sel")
        H_T0 = pers.tile([128, T], f32r, tag="H_T0")
        H_T1 = pers.tile([64, T], f32r, tag="H_T1")
        hgs = pers.tile([128, 512], f32, tag="hgs")

        # ---- right-side transients: xTr (f32r copy of xT), fp32 xT quarters ----
        xtr_pool = tc.alloc_tile_pool(name="xtrp", bufs=1, side="right")
        xTr = xtr_pool.tile([128, DCH, T], f32r, tag="xTr")

        # scores (fp32) + xTr copies, token quarters to bound SBUF
        xtp = tc.alloc_tile_pool(name="xtp", bufs=1, side="right")
        for q in range(4):
            xt_q = xtp.tile([128, DCH, 256], f32, tag="xt_q")
            nc.sync.dma_start(xt_q[:], xT_h[:, :, q * 256:(q + 1) * 256])
            for j in range(DCH):
                nc.any.tensor_copy(xTr[:, j, q * 256:(q + 1) * 256], xt_q[:, j, :])
            for ii in range(2):
                i = q * 2 + ii
                sc_ps = ps_mx.tile([128, E], f32, tag="mx")
                for j in range(DCH):
                    nc.tensor.matmul(sc_ps[:], xt_q[:, j, ii * 128:(ii + 1) * 128],
                                     gw[:, j, :], start=(j == 0), stop=(j == DCH - 1))
                nc.any.tensor_copy(logits[:, i, :], sc_ps[:])
        xtp.release()

        # ---- shared expert up/gate (PE) — overlaps top-k (DVE) ----
        sgsup = tc.alloc_tile_pool(name="sgsup", bufs=1, side="right")
        sgt = sgsup.tile([128, DCH, FSHL], f32r, tag="sgt")
        nc.sync.dma_start(sgt[:], sg_h[:])
        sut = sgsup.tile([128, DCH, FSHL], f32r, tag="sut")
        nc.sync.dma_start(sut[:], su_h[:])
        for hc, hofs, hsz, htile in ((0, 0, 128, H_T0), (1, 128, 64, H_T1)):
            for th in range(2):
                hg_ps = ps_y.tile([hsz, 512], f32, tag="y")
                hu_ps = ps_y.tile([hsz, 512], f32, tag="y")
                for j in range(DCH):
                    st = (j == 0)
                    sp = (j == DCH - 1)
                    nc.tensor.matmul(hg_ps[:], sgt[:, j, hofs:hofs + hsz],
                                     xTr[:, j, th * 512:(th + 1) * 512], start=st, stop=sp)
                    nc.tensor.matmul(hu_ps[:], sut[:, j, hofs:hofs + hsz],
                                     xTr[:, j, th * 512:(th + 1) * 512], start=st, stop=sp)
                nc.scalar.activation(hgs[:hsz, :], hg_ps[:], AF.Silu)
                nc.vector.tensor_tensor(htile[:, th * 512:(th + 1) * 512],
                                        hgs[:hsz, :], hu_ps[:], op=ALU.mult)
        sgsup.release()
        xtr_pool.release()

        # ---- top-k on logits (DVE) ----
        cur = pers.tile([128, TCH, E], f32, tag="cur")
        nc.vector.tensor_copy(cur[:], logits[:])
        nc.vector.memset(sel[:], 0.0)
        mx = pers.tile([128, TCH], f32, tag="mxt")
        eq = pers.tile([128, TCH, E], f32, tag="eq")
        tktmp = pers.tile([128, TCH, E], f32, tag="tktmp")
        for _ in range(K):
            nc.vector.tensor_reduce(mx[:], cur[:], axis=AX.X, op=ALU.max)
            nc.vector.tensor_tensor(eq[:], cur[:], mx[:].broadcast_to([128, TCH, E]),
                                    op=ALU.is_ge)
            nc.vector.tensor_tensor(sel[:], sel[:], eq[:], op=ALU.add)
            # cur = cur - cur*eq - eq*1e30  (mask selected down to ~-1e30)
            nc.vector.tensor_tensor(tktmp[:], cur[:], eq[:], op=ALU.mult)
            nc.vector.tensor_tensor(cur[:], cur[:], tktmp[:], op=ALU.subtract)
            nc.vector.tensor_scalar(tktmp[:], eq[:], 1e30, None, op0=ALU.mult)
            nc.vector.tensor_tensor(cur[:], cur[:], tktmp[:], op=ALU.subtract)

        # combine weights: w = sigmoid(logit)*sel / sum(sigmoid*sel)
        sig = pers.tile([128, TCH, E], f32, tag="sig")
        nc.scalar.activation(sig[:], logits[:], AF.Sigmoid)
        ssel = pers.tile([128, TCH, E], f32, tag="ssel")
        nc.vector.tensor_tensor(ssel[:], sig[:], sel[:], op=ALU.mult)
        den = pers.tile([128, TCH], f32, tag="den")
        nc.vector.tensor_reduce(den[:], ssel[:], axis=AX.X, op=ALU.add)
        rec = pers.tile([128, TCH], f32, tag="rec")
        nc.vector.reciprocal(rec[:], den[:])
        nc.vector.tensor_tensor(wfull[:], ssel[:], rec[:].broadcast_to([128, TCH, E]),
                                op=ALU.mult)
        nc.vector.tensor_copy(wr[:], wfull[:])
        nc.vector.tensor_copy(selr[:], sel[:, :, 0:ELOC])

        # ---- slot positions: exclusive cumsum over tokens of sel (cols 0..3) ----
        t4 = pers.tile([128, ELOC], f32, tag="t4")
        for i in range(TCH):
            pos_ps = ps_mx.tile([128, ELOC], f32, tag="mx")
            for j in range(i):
                nc.tensor.matmul(pos_ps[:], ones128[:], selr[:, j, :],
                                 start=(j == 0), stop=False)
            nc.tensor.matmul(pos_ps[:], ustrict[:], selr[:, i, :],
                             start=(i == 0), stop=True)
            nc.vector.tensor_scalar(t4[:], pos_ps[:], 1.0, None, op0=ALU.add)
            nc.vector.tensor_tensor(t4[:], t4[:], sel[:, i, 0:ELOC], op=ALU.mult)
            nc.vector.tensor_scalar(pos_sel[:, i, :], t4[:], 1.0, None, op0=ALU.subtract)

        # ---- per-expert compact MLP ----
        setw_pool = tc.alloc_tile_pool(name="setwp", bufs=1)
        setw = [[setw_pool.tile([128, T], f32r, tag=f"setw{e}_{cc}", name=f"setw{e}_{cc}")
                 for cc in range(2)] for e in range(ELOC)]
        yspill = [dram.tile([128, 2, D], f32r, tag=f"ysp{e}", name=f"ysp{e}") for e in range(ELOC)]

        se_pool = tc.alloc_tile_pool(name="sep", bufs=1)
        xet_pool = tc.alloc_tile_pool(name="xetp", bufs=1)
        a_pool = tc.alloc_tile_pool(name="apool", bufs=1)
        wgu_pool = tc.alloc_tile_pool(name="wgup", bufs=4)
        wd_pool = tc.alloc_tile_pool(name="wdp", bufs=2)
        ysb_pool = tc.alloc_tile_pool(name="ysbp", bufs=2)
        small = tc.alloc_tile_pool(name="smallp", bufs=2)

        for e in range(ELOC):
            # one-hot dispatch S_e[t, c] = (pos_sel[t, e] == c)
            S_e = se_pool.tile([128, TCH, C], f32r, tag="se")
            for i in range(TCH):
                nc.vector.tensor_scalar(S_e[:, i, :], iotaC[:], pos_sel[:, i, e:e + 1],
                                        None, op0=ALU.is_equal)
            # per-slot combine weight: w_slot[c] = sum_t S_e[t,c] * w[t,e]
            wslot = [None, None]
            for cc in range(2):
                ws_ps = ps_mx.tile([128, ELOC], f32, tag="mx", name="ws_ps")
                for i in range(TCH):
                    nc.tensor.matmul(ws_ps[:], S_e[:, i, cc * 128:(cc + 1) * 128],
                                     wr[:, i, 0:ELOC], start=(i == 0), stop=(i == TCH - 1))
                wslot[cc] = small.tile([128, 1], f32, tag="wslot", name=f"wslot{cc}")
                nc.any.tensor_copy(wslot[cc][:], ws_ps[:, e:e + 1])
            # S_eT (transposed, weight-scaled): setw[e][cc][c, t]
            for cc in range(2):
                for i in range(TCH):
                    tr_ps = ps_mx.tile([128, 128], f32r, tag="mx", name="tr_ps")
                    nc.tensor.transpose(tr_ps[:], S_e[:, i, cc * 128:(cc + 1) * 128], ident[:])
                    nc.vector.tensor_scalar(setw[e][cc][:, i * 128:(i + 1) * 128],
                                            tr_ps[:], wslot[cc][:, 0:1], None, op0=ALU.mult)
            # gather: XeT[d, c] = sum_t x[t, d] S_e[t, c]
            XeT = xet_pool.tile([128, DCH, C], f32r, tag="xet")
            for j in range(DCH):
                g_ps = ps_mx.tile([128, C], f32, tag="mx")
                for i in range(TCH):
                    nc.tensor.matmul(g_ps[:], xtok[:, i, j * 128:(j + 1) * 128],
                                     S_e[:, i, :], start=(i == 0), stop=(i == TCH - 1))
                nc.any.tensor_copy(XeT[:, j, :], g_ps[:])
            # up/gate + silu*u -> A[f, c]
            A = a_pool.tile([128, FCH, C], f32r, tag="a")
            for f in range(FCH):
                wgf = [wgu_pool.tile([128, DCH // 2, 128], f32r, tag="wgu", name=f"wgf{h}") for h in range(2)]
                wuf = [wgu_pool.tile([128, DCH // 2, 128], f32r, tag="wgu", name=f"wuf{h}") for h in range(2)]
                for h in range(2):
                    js = slice(h * (DCH // 2), (h + 1) * (DCH // 2))
                    nc.sync.dma_start(wgf[h][:], wg_h[e, f][:, js, :])
                    nc.sync.dma_start(wuf[h][:], wu_h[e, f][:, js, :])
                G_ps = ps_gu.tile([128, C], f32, tag="gu")
                U_ps = ps_gu.tile([128, C], f32, tag="gu")
                for j in range(DCH):
                    st = (j == 0)
                    sp = (j == DCH - 1)
                    h, jj = divmod(j, DCH // 2)
                    nc.tensor.matmul(G_ps[:], wgf[h][:, jj, :], XeT[:, j, :], start=st, stop=sp)
                    nc.tensor.matmul(U_ps[:], wuf[h][:, jj, :], XeT[:, j, :], start=st, stop=sp)
                gs = small.tile([128, C], f32, tag="gs")
                nc.scalar.activation(gs[:], G_ps[:], AF.Silu)
                nc.vector.tensor_tensor(A[:, f, :], gs[:], U_ps[:], op=ALU.mult)
            # down: Y[c, d] = sum_f A[f, c] wd[f, d]  -> spill to DRAM
            for dtile in range(4):
                wds = [wd_pool.tile([128, FCH // 2, 512], f32r, tag="wd", name=f"wds{h}") for h in range(2)]
                for h in range(2):
                    fs = slice(h * (FCH // 2), (h + 1) * (FCH // 2))
                    nc.sync.dma_start(wds[h][:], wd_h[e, dtile][:, fs, :])
                for cc in range(2):
                    y_ps = ps_y.tile([128, 512], f32, tag="y")
                    for f in range(FCH):
                        h, ff = divmod(f, FCH // 2)
                        nc.tensor.matmul(y_ps[:], A[:, f, cc * 128:(cc + 1) * 128],
                                         wds[h][:, ff, :], start=(f == 0), stop=(f == FCH - 1))
                    ysb = ysb_pool.tile([128, 512], f32r, tag="ysb")
                    nc.any.tensor_copy(ysb[:], y_ps[:])
                    nc.sync.dma_start(yspill[e][:, cc, dtile * 512:(dtile + 1) * 512], ysb[:])

        small.release()
        ysb_pool.release()
        wd_pool.release()
        wgu_pool.release()
        a_pool.release()
        xet_pool.release()
        se_pool.release()

        # ---- final accumulation: routed (scatter) + shared down, token-major ----
        yinp = tc.alloc_tile_pool(name="yinp", bufs=2)
        sdp = tc.alloc_tile_pool(name="sdp", bufs=1)
        osbp = tc.alloc_tile_pool(name="osbp", bufs=4)
        sd0 = sdp.tile([128, D], f32r, tag="sd0")
        nc.sync.dma_start(sd0[:], sd0_h[:])
        sd1 = sdp.tile([64, D], f32r, tag="sd1")
        nc.sync.dma_start(sd1[:], sd1_h[:])
        for dtile in range(4):
            yins = []
            for e in range(ELOC):
                yin = yinp.tile([128, 2, 512], f32r, tag=f"yin{e}", name=f"yin{e}")
                nc.sync.dma_start(yin[:], yspill[e][:, :, dtile * 512:(dtile + 1) * 512])
                yins.append(yin)
            for i in range(TCH):
                r_ps = ps_y.tile([128, 512], f32, tag="y")
                first = True
                for e in range(ELOC):
                    for cc in range(2):
                        nc.tensor.matmul(r_ps[:], setw[e][cc][:, i * 128:(i + 1) * 128],
                                         yins[e][:, cc, :], start=first, stop=False)
                        first = False
                nc.tensor.matmul(r_ps[:], H_T0[:, i * 128:(i + 1) * 128],
                                 sd0[:, dtile * 512:(dtile + 1) * 512],
                                 start=False, stop=False)
                nc.tensor.matmul(r_ps[:], H_T1[:, i * 128:(i + 1) * 128],
                                 sd1[:, dtile * 512:(dtile + 1) * 512],
                                 start=False, stop=True)
                osb = osbp.tile([128, 512], f32, tag="osb")
                nc.any.tensor_copy(osb[:], r_ps[:])
                nc.sync.dma_start(out_h[i * 128:(i + 1) * 128,
                                        dtile * 512:(dtile + 1) * 512], osb[:])
        osbp.release()
        sdp.release()
        yinp.release()
        setw_pool.release()

    nc.compile()
    return nc


def _get_compiled():
    global _COMPILED
    if _COMPILED is None:
        _COMPILED = _build()
    return _COMPILED


def _prep_in_maps(inputs):
    x = np.ascontiguousarray(np.asarray(inputs["hidden_states"], np.float32).reshape(T, D))
    gate_w = np.asarray(inputs["gate_w"], np.float32)
    wg = np.asarray(inputs["wg"], np.float32)
    wu = np.asarray(inputs["wu"], np.float32)
    wd = np.asarray(inputs["wd"], np.float32)
    sg = np.asarray(inputs["sg"], np.float32)
    su = np.asarray(inputs["su"], np.float32)
    sd = np.asarray(inputs["sd"], np.float32)

    xtok_t = x.reshape(TCH, 128, D).transpose(1, 0, 2).copy()
    xT_t = x.T.reshape(DCH, 128, T).transpose(1, 0, 2).copy()

    in_maps = []
    for c in range(NCORES):
        lo = ELOC * c
        perm = list(range(lo, lo + ELOC)) + [e for e in range(E) if not lo <= e < lo + ELOC]
        gw_t = gate_w[:, perm].reshape(DCH, 128, E).transpose(1, 0, 2).copy()
        wg_t = wg[lo:lo + ELOC].reshape(ELOC, DCH, 128, FCH, 128).transpose(0, 3, 2, 1, 4).copy()
        wu_t = wu[lo:lo + ELOC].reshape(ELOC, DCH, 128, FCH, 128).transpose(0, 3, 2, 1, 4).copy()
        wd_t = wd[lo:lo + ELOC].reshape(ELOC, FCH, 128, 4, 512).transpose(0, 3, 2, 1, 4).copy()
        hs = slice(c * FSHL, (c + 1) * FSHL)
        sg_t = sg[:, hs].reshape(DCH, 128, FSHL).transpose(1, 0, 2).copy()
        su_t = su[:, hs].reshape(DCH, 128, FSHL).transpose(1, 0, 2).copy()
        sdl = sd[hs, :]
        in_maps.append({
            "xtok": xtok_t, "xT": xT_t, "gw": gw_t,
            "wg": wg_t, "wu": wu_t, "wd": wd_t,
            "sg": sg_t, "su": su_t,
            "sd0": sdl[0:128].copy(), "sd1": sdl[128:FSHL].copy(),
        })
    return in_maps


def run_raw(inputs, trace=False, tmpdir=None):
    from concourse.bass_utils import run_bass_kernel_spmd
    nc = _get_compiled()
    in_maps = _prep_in_maps(inputs)
    return run_bass_kernel_spmd(nc, in_maps, list(range(NCORES)),
                                trace=trace, tmpdir=tmpdir)


def kernel(**inputs) -> np.ndarray:
    res = run_raw(inputs)
    out = np.zeros((T, D), np.float32)
    for r in res.results:
        out += r["out"]
    return out.reshape(1, T, D)
